# revision 36
# baseline (speedup 1.0000x reference)
"""BNN MNIST MLP on 8 Trainium2 NeuronCores — pure data parallel.

Model (inference): x[B,784] -> relu(x @ sign(W1)) -> BN1 -> sign ->
@ sign(W2) relu BN2 sign -> @ sign(W3) -> softmax.

Key transformations:
  * BN(relu(h)) >= 0  <=>  h >= t  (per-feature threshold t, since BN scale>0),
    so each binarize step is one ScalarE Sign(h - t) op straight from PSUM.
  * Layer-1 needs fp32-class precision (sign margins ~2.5e-5): x is split on
    host into fp16 hi + lo halves (same total bytes as fp32); both halves are
    stacked into one [1568, B] feature-major tensor and the matmul contracts
    over all 1568 rows against [sign(W1); sign(W1)] — fp16 runs at 1 PE
    cycle/row vs 4 for native fp32, and PSUM accumulates in fp32.
  * x ships pre-transposed (feature-major) per core; DMA granularity is 2048
    batch columns (4 KB contiguous per partition line -> near-line-rate SDMA
    engine efficiency) while compute runs on 1024-column slabs.
  * The hidden width (50) uses only half the PE array columns, so the two
    512-row groups of each compute slab run CONCURRENTLY via column tiling
    (out partitions 0-49 / 64-113) — halving layer-1 streaming time.
  * Layer 3 is computed feature-major: logits[10, 512] = w3^T @ s2 as ONE
    column-tiled matmul pair per slab (vs 16 tiny batch-major matmuls), exp
    runs on the PSUM tile, and the unnormalized exp ships feature-major; the
    softmax row-sum division happens on host during unsharding (0.65 M
    elements, negligible next to the input repack).
  * Lag-1 software pipeline: tick p emits A(p), B(p-1), CD(p-1) so only one
    slab's worth of dependent work trails the final DMA (~3 us tail).
"""
import numpy as np

import concourse.mybir as mybir
from concourse import bacc
from concourse.tile import TileContext
from concourse.bass_utils import run_bass_kernel_spmd

F32 = mybir.dt.float32
F16 = mybir.dt.float16

B = 65536
NCORES = 8
PER = B // NCORES          # 8192 rows per core
SLAB = 1024                # rows per compute slab
NSLAB = PER // SLAB        # 8
GRP = 512                  # rows per PSUM group (one matmul N)
# DMA blocks: three 2048-col blocks (4 KB partition lines, 512 KB
# transfers) cover slabs 0-5; the final 2048 cols are split per-slab
# (1024-col blocks, 256 KB transfers) so slab 6's dependent chain overlaps
# slab 7's streaming and only one ~5 us chain trails the stream.
DBLK = [(0, 2048), (2048, 2048), (4096, 2048), (6144, 1024), (7168, 1024)]
DSLAB_OF = [0, 0, 1, 1, 2, 2, 3, 4]       # compute slab -> DMA block
K = 784
K2 = 2 * K                 # hi+lo stacked contraction length (1568)
KC = 128                   # contraction chunk (full partition width)
NKC = (K2 + KC - 1) // KC  # 13 chunks: 12 x 128 + 1 x 32
NCLS = 10
NHID = 50

EPS = 1e-3

_CACHE = {}


def _build():
    nc = bacc.Bacc("TRN2", target_bir_lowering=False, debug=False,
                   num_devices=NCORES)

    xcat = nc.dram_tensor("xcat", [K2, PER], F16, kind="ExternalInput").ap()
    # all fp16 consts packed in one blob: w1 chunks at cols [50c, 50c+50),
    # w2 at [650, 700), w3 at [700, 710)
    cb16 = nc.dram_tensor("cb16", [128, NHID * NKC + NHID + NCLS], F16,
                          kind="ExternalInput").ap()
    # fp32 consts: col 0 = -T1, col 1 = -T2, col 2 = +T2 (all replicated at
    # partition offset 64 for the column-tiled pair)
    cb32 = nc.dram_tensor("cb32", [128, 3], F32, kind="ExternalInput").ap()
    # unnormalized exp(logits), feature-major: rows 0-9 = classes of batch
    # halves 0, rows 64-73 = classes of batch halves 1 (one [74,512] store
    # per slab; rows 10-63 are garbage the host ignores)
    out = nc.dram_tensor("out", [64 + NCLS, NSLAB * GRP], F32,
                         kind="ExternalOutput").ap()

    kc = [min(KC, K2 - c * KC) for c in range(NKC)]

    with TileContext(nc) as tc:
        with (
            tc.tile_pool(name="consts", bufs=1) as cpool,
            tc.tile_pool(name="xin", bufs=2) as xpool,
            tc.tile_pool(name="mid", bufs=3) as mpool,
            tc.tile_pool(name="fin", bufs=8) as fpool,
            tc.tile_pool(name="ps1", bufs=2, space="PSUM") as psA,
            tc.tile_pool(name="ps2", bufs=2, space="PSUM") as psB,
            tc.tile_pool(name="ps3", bufs=2, space="PSUM") as psC,
        ):
            # consts go through SWDGE (gpsimd) so the HWDGE queues start
            # streaming x immediately
            cb16t = cpool.tile([128, NHID * NKC + NHID + NCLS], F16, tag="cb16")
            nc.gpsimd.dma_start(cb16t[:], cb16[:, :])
            cb32t = cpool.tile([128, 3], F32, tag="cb32")
            nc.gpsimd.dma_start(cb32t[:], cb32[:, :])
            w1t = [cb16t[0:kc[c], c * NHID:(c + 1) * NHID] for c in range(NKC)]
            w2t = cb16t[0:NHID, NKC * NHID:NKC * NHID + NHID]
            w3t = cb16t[0:NHID, NKC * NHID + NHID:NKC * NHID + NHID + NCLS]
            w2t64 = cb16t[64:64 + NHID, NKC * NHID:NKC * NHID + NHID]
            w3t64 = cb16t[64:64 + NHID,
                          NKC * NHID + NHID:NKC * NHID + NHID + NCLS]
            nt1t = cb32t[0:64 + NHID, 0:1]
            nt2t = cb32t[0:64 + NHID, 1:2]
            pt2t = cb32t[0:64 + NHID, 2:3]

            xt = {}
            s1t = {}
            s2t = {}
            eot = {}

            def emit_loads(d):
                b0, w = DBLK[d]
                tg = "x" if w == 2048 else "xs"
                xt[d] = []
                for c in range(NKC):
                    t_ = xpool.tile([kc[c], w], F16, tag=f"{tg}_{c}",
                                    name=f"x_{d}_{c}")
                    # all loads on the Sync HWDGE ring: the Scalar engine
                    # stays a pure-ACT engine, so Tile's DMA bookkeeping
                    # waits never block sign/exp (SWDGE descriptor gen is too
                    # slow to carry half the stream; one HWDGE ring can feed
                    # all 16 SDMA engines)
                    nc.sync.dma_start(t_[:], xcat[c * KC:c * KC + kc[c], b0:b0 + w])
                    xt[d].append(t_)

            ps1t = {}

            def stageA_mm(p, c):
                # one compute slab = 1024 rows = 2 groups of 512, run
                # CONCURRENTLY on the PE via column tiling: group 0 on array
                # columns 0-63 (out partitions 0-49), group 1 on columns
                # 64-127 (out partitions 64-113).
                d = DSLAB_OF[p]
                h = p * SLAB - DBLK[d][0]
                if c == 0:
                    ps1t[p] = psA.tile([128, GRP], F32, tag="ps1",
                                       name=f"ps1_{p}")
                ps1 = ps1t[p]
                xc = xt[d][c]
                nc.tensor.matmul(ps1[0:NHID, :], w1t[c],
                                 xc[:, h:h + GRP],
                                 start=(c == 0), stop=(c == NKC - 1),
                                 skip_group_check=True)
                nc.tensor.matmul(ps1[64:64 + NHID, :], w1t[c],
                                 xc[:, h + GRP:h + 2 * GRP],
                                 start=(c == 0), stop=(c == NKC - 1),
                                 skip_group_check=True)

            def stageA_sign(p):
                s1 = mpool.tile([64 + NHID, GRP], F16, tag="s1", name=f"s1_{p}")
                nc.scalar.sign(s1[:], ps1t[p][0:64 + NHID, :], bias=nt1t)
                s1t[p] = (s1[0:NHID, :], s1[64:64 + NHID, :])

            def stageA(p):
                for c in range(NKC):
                    stageA_mm(p, c)
                stageA_sign(p)

            def stageB(p, dve_sign=False):
                ps2 = psB.tile([128, GRP], F32, tag="ps2")
                sa, sb = s1t[p]
                nc.tensor.matmul(ps2[0:NHID, :], w2t, sa,
                                 start=True, stop=True, skip_group_check=True)
                nc.tensor.matmul(ps2[64:64 + NHID, :], w2t64, sb,
                                 start=True, stop=True, skip_group_check=True)
                s2 = mpool.tile([64 + NHID, GRP], F16, tag="s2", name=f"s2_{p}")
                nc.scalar.sign(s2[:], ps2[0:64 + NHID, :], bias=nt2t)
                s2t[p] = (s2[0:NHID, :], s2[64:64 + NHID, :])

            def stageCD(p):
                # Layer 3 feature-major: logits[10, 512] = w3^T @ s2 as one
                # column-tiled pair; exp straight off PSUM; store 2 KB lines.
                ps3 = psC.tile([128, GRP], F32, tag="ps3", name=f"ps3_{p}")
                sa, sb = s2t[p]
                nc.tensor.matmul(ps3[0:NCLS, :], w3t, sa,
                                 start=True, stop=True, skip_group_check=True)
                nc.tensor.matmul(ps3[64:64 + NCLS, :], w3t64, sb,
                                 start=True, stop=True, skip_group_check=True)
                eo = fpool.tile([64 + NCLS, GRP], F32, tag="eo", name=f"eo_{p}")
                nc.scalar.activation(eo[:], ps3[0:64 + NCLS, :],
                                     mybir.ActivationFunctionType.Exp)
                eot[p] = eo

            # steady state: B(p-1)/CD(p-1) are emitted BEFORE A(p) so during
            # the stream the dependent chain of slab p-1 runs inside A(p)'s
            # DMA-arrival slack.  The last two slabs are chunk-interleaved
            # with their signs detached, so BOTH final PSUM accumulations
            # complete right at stream end and only one short chain
            # (signs/B/CD/exp/stores for 6 and 7, pipelined across ACT and
            # PE) trails the final DMA.
            emit_loads(0)
            emit_loads(1)
            for p in range(NSLAB - 2):
                if p >= 1:
                    stageB(p - 1)
                    stageCD(p - 1)
                stageA(p)
                if p == 0:
                    emit_loads(2)
                elif p == 1:
                    emit_loads(3)
                elif p == 2:
                    emit_loads(4)
            stageB(NSLAB - 3)      # B(5)
            stageCD(NSLAB - 3)     # CD(5)
            stageA(NSLAB - 2)      # A(6) — block d3a ends ~9us before the
            stageB(NSLAB - 2)      # B(6)   stream, so slab 6's whole chain
            stageCD(NSLAB - 2)     # CD(6)  overlaps d3b's streaming
            stageA(NSLAB - 1)      # A(7)
            stageB(NSLAB - 1)      # B(7)
            stageCD(NSLAB - 1)     # CD(7)
            # ALL stores are emitted after ALL loads: the HWDGE completion
            # lanes are assigned round-robin in emission order, so a store
            # stalled on its exp-wait would freeze its lane and block every
            # later load sharing it.  Emitted last, stores 0-5 are already
            # satisfied and drain back-to-back while the tail computes.
            for p in range(NSLAB):
                nc.sync.dma_start(out[0:64 + NCLS, p * GRP:(p + 1) * GRP],
                                  eot[p][:])

    nc.compile()
    return nc


def _prep_host(inputs, W1, W2, W3, g1, b1, m1, v1, g2, b2, m2, v2):
    x = np.ascontiguousarray(inputs.reshape(B, K).astype(np.float32, copy=False))
    xhi = x.astype(np.float16)
    xlo = (x - xhi.astype(np.float32)).astype(np.float16)

    w1b = np.where(W1 >= 0, 1.0, -1.0).astype(np.float16)
    w2b = np.where(W2 >= 0, 1.0, -1.0).astype(np.float16)
    w3b = np.where(W3 >= 0, 1.0, -1.0).astype(np.float16)

    a1 = g1.astype(np.float64) / np.sqrt(v1.astype(np.float64) + EPS)
    c1 = b1.astype(np.float64) - a1 * m1.astype(np.float64)
    t1 = -c1 / a1
    T1 = np.where(t1 > 0, t1, -1e30).astype(np.float32)
    a2 = g2.astype(np.float64) / np.sqrt(v2.astype(np.float64) + EPS)
    c2 = b2.astype(np.float64) - a2 * m2.astype(np.float64)
    t2 = -c2 / a2
    T2 = np.where(t2 > 0, t2, -1e30).astype(np.float32)

    w1cat = np.vstack([w1b, w1b])
    cb16 = np.zeros((128, NHID * NKC + NHID + NCLS), dtype=np.float16)
    for c in range(NKC):
        n = min(KC, K2 - c * KC)
        cb16[:n, c * NHID:(c + 1) * NHID] = w1cat[c * KC:c * KC + n]
    cb16[:NHID, NKC * NHID:NKC * NHID + NHID] = w2b
    cb16[:NHID, NKC * NHID + NHID:] = w3b
    cb16[64:64 + NHID, NKC * NHID:NKC * NHID + NHID] = w2b
    cb16[64:64 + NHID, NKC * NHID + NHID:] = w3b
    cb32 = np.zeros((128, 3), dtype=np.float32)
    cb32[:NHID, 0] = -T1
    cb32[64:64 + NHID, 0] = -T1
    cb32[:NHID, 1] = -T2
    cb32[64:64 + NHID, 1] = -T2
    cb32[:NHID, 2] = T2
    cb32[64:64 + NHID, 2] = T2
    shared = {"cb16": cb16, "cb32": cb32}
    in_maps = []
    for c in range(NCORES):
        sl = slice(c * PER, (c + 1) * PER)
        m = dict(shared)
        xc = np.empty((K2, PER), dtype=np.float16)
        xc[:K] = xhi[sl].T
        xc[K:] = xlo[sl].T
        m["xcat"] = xc
        in_maps.append(m)
    return in_maps


def kernel(**inputs):
    if "nc" not in _CACHE:
        _CACHE["nc"] = _build()
    nc = _CACHE["nc"]
    inputs = {k: np.asarray(v) for k, v in inputs.items()}
    in_maps = _prep_host(**inputs)
    res = run_bass_kernel_spmd(nc, in_maps, core_ids=list(range(NCORES)))
    # out is [74, 4096] per core: rows 0-9 = classes for batch cols 0-511 of
    # each slab, rows 64-73 = classes for batch cols 512-1023
    parts = []
    for r in res.results:
        o = r["out"].reshape(64 + NCLS, NSLAB, GRP)
        e = np.empty((NSLAB, 2 * GRP, NCLS), dtype=np.float32)
        e[:, :GRP, :] = o[0:NCLS].transpose(1, 2, 0)
        e[:, GRP:, :] = o[64:64 + NCLS].transpose(1, 2, 0)
        parts.append(e.reshape(PER, NCLS))
    e = np.concatenate(parts, axis=0)
    return (e / e.sum(axis=1, keepdims=True)).astype(np.float32)
